# revision 19
# baseline (speedup 1.0000x reference)
"""BiLSTM-CRF loss kernel for Trainium2 (8 NeuronCores, Bass/Tile).

Strategy (see spec sharding_hint; we deviate deliberately):
  The LSTM recurrence is latency-bound per step (the whole 2048x512 W_hh must
  stream through the PE array every step regardless of local batch size), so
  data-parallel batch sharding does not speed it up. Instead: core 0 runs the
  forward-direction LSTM on the FULL batch, core 1 runs the backward direction
  (as a forward loop over a sequence-reversed gather order). Cores 2-7 run the
  same (uniform) program on dummy data and contribute zeros to the single
  AllReduce that combines the two cores' projection partials into the CRF
  feature tensor. The CRF forward pass + gold-path emit score then run
  (redundantly) on every core; host reads core 0's result.

Program is fully uniform across cores -- all role differences are input data.
"""
import os, sys

for _p in ("/opt/trn_rl_repo", "/root/.axon_site/_ro/trn_rl_repo"):
    if os.path.isdir(_p) and _p not in sys.path:
        sys.path.append(_p)

import numpy as np
import ml_dtypes

from concourse import bass, bacc, mybir, tile
from concourse.bass_utils import run_bass_kernel_spmd

AF = mybir.ActivationFunctionType
ALU = mybir.AluOpType
BF16 = mybir.dt.bfloat16
FP32 = mybir.dt.float32
I32 = mybir.dt.int32

# model dims (hardcoded per contract)
S, B, V, E, H, T = 256, 32, 50000, 512, 512, 30
START, STOP = 28, 29
G4 = 4 * H          # 2048 gates
NMT = G4 // 128     # 16 gate tiles
NKC = H // 128      # 4 h-feature chunks
NEC = E // 128      # 4 embedding-feature chunks
CNK = 32            # recurrence steps per pipeline chunk
NCHUNK = S // CNK   # 8
RENORM_EVERY = 8
W8_SCALE = 256.0


def _install_tile_drain_patch():
    """This container's walrus rejects multi-wait Drain instructions
    ("Too many sync wait commands"); move the TileContext tail-drain waits
    onto separate wait_ge instructions."""
    def _patched(self, tick_clock, wait_clock):
        nop = self.nc.sync.nop()
        wait_clock.add_sem_waits(nop.ins, tile.ScopedClock({None: tick_clock.global_clock}))
        si = nop.ins.sync_info
        waits = list(si.on_wait) if si is not None else []
        num2handle = {h.num: h for h in self.sems.allocated().values()}
        if si is not None:
            si.on_wait = waits[:1]
        for w in waits[1:]:
            self.nc.sync.wait_ge(num2handle[w.id], w.wait_value)
        self.nc.sync.drain()
        self.nc.all_engine_barrier()
        popped = self.nc._tile_sem_poison_stack.pop()
        assert popped is self._sem_poison
        self.nc.clear_and_free_semaphores(list(self.sems.allocated().values()))
        self.nc.all_engine_barrier()

    tile.TileContext._drain_and_barrier = _patched


_install_tile_drain_patch()


def build_program(n_cores=8, s=S, body_reps=1, phases="ABCDR", w8=False):
    """One uniform program; all per-core behavior differences come from data.
    body_reps>1 repeats the whole computation serially (timing amplification).
    phases: subset of "ABCD" for truncated builds (timing attribution)."""
    nchunk = s // CNK
    ntok = s * B

    nc = bacc.Bacc("TRN2", target_bir_lowering=False, debug=False,
                   num_devices=n_cores)
    # ---- inputs (per-core data) ----
    emb_bf = nc.dram_tensor("emb_bf", [V, E], BF16, kind="ExternalInput")
    sidx = nc.dram_tensor("sidx", [ntok, 1], I32, kind="ExternalInput")
    whh_dt = mybir.dt.float8e4 if w8 else BF16
    wihT = nc.dram_tensor("wihT", [NEC, 128, G4], whh_dt, kind="ExternalInput") # [kc][p][gate]
    whhT = nc.dram_tensor("whhT", [NKC, 128, G4], whh_dt, kind="ExternalInput")
    bsum = nc.dram_tensor("bsum", [128, NMT], FP32, kind="ExternalInput")       # b_ih+b_hh, col mt
    w8inv = nc.dram_tensor("w8inv", [128, 1], FP32, kind="ExternalInput")       # 1/w8 scale (1.0 if bf16)
    ident = nc.dram_tensor("ident", [128, 128], BF16, kind="ExternalInput")     # diag(w8 scale)
    h_init = nc.dram_tensor("h_init", [128, 128], BF16, kind="ExternalInput")   # (p, hc*32+b)
    c_init = nc.dram_tensor("c_init", [128, 128], FP32, kind="ExternalInput")
    woutA = nc.dram_tensor("woutA", [NKC, 128, T], BF16, kind="ExternalInput")  # lhsT tiles, fwd-order hT
    woutB = nc.dram_tensor("woutB", [NKC, 128, T], BF16, kind="ExternalInput")  # lhsT tiles, rev-order hT
    bout = nc.dram_tensor("bout", [T, 1], FP32, kind="ExternalInput")
    ET = nc.dram_tensor("ET", [T, T], FP32, kind="ExternalInput")               # lhsT[j,t]=exp(trans[t,j])
    X0 = nc.dram_tensor("X0", [T, B], FP32, kind="ExternalInput")               # exp(fv0)
    estopT = nc.dram_tensor("estopT", [T, 1], FP32, kind="ExternalInput")       # exp(trans[STOP,:])
    rn = nc.dram_tensor("rn", [T, 2], FP32, kind="ExternalInput")               # col0=1, col1=renorm
    onesT = nc.dram_tensor("onesT", [T, 1], FP32, kind="ExternalInput")
    gmask = nc.dram_tensor("gmask", [T, ntok], BF16, kind="ExternalInput")      # onehot(tags) mask

    # ---- outputs ----
    out_z = nc.dram_tensor("out_z", [1, B], FP32, kind="ExternalOutput")
    out_emit = nc.dram_tensor("out_emit", [1, B], FP32, kind="ExternalOutput")

    with tile.TileContext(nc) as tc:
        with tc.tile_pool(name="dram", bufs=1, space="DRAM") as dram, \
             tc.tile_pool(name="const", bufs=1) as const:
            # DRAM scratch
            featT_loc = dram.tile([T, ntok], FP32)
            featT_shr = dram.tile([T, ntok], FP32)
            # all h states stay resident in SBUF: (128, [s][hc2][b]) bf16
            hSB = const.tile([128, s * 128], BF16)

            bsum_sb = const.tile([128, NMT], FP32)
            nc.sync.dma_start(out=bsum_sb[:], in_=bsum[:])
            w8inv_sb = const.tile([128, 1], FP32)
            nc.sync.dma_start(out=w8inv_sb[:], in_=w8inv[:])
            ident_sb = const.tile([128, 128], BF16)
            nc.sync.dma_start(out=ident_sb[:], in_=ident[:])

            for _rep in range(body_reps):
              # ============ phase A+B: gather -> transpose -> xw -> recurrence ============
              with nc.named_scope("phAB"), \
                   tc.tile_pool(name="wpool", bufs=1) as wpool, \
                   tc.tile_pool(name="gath", bufs=3) as gath, \
                   tc.tile_pool(name="xwp", bufs=2) as xwp, \
                   tc.tile_pool(name="rec", bufs=2) as rec, \
                   tc.tile_pool(name="psA", bufs=2, space="PSUM") as psum:
                  wih_sb = wpool.tile([128, NEC * G4], whh_dt)
                  whh_sb = wpool.tile([128, NKC * G4], whh_dt)
                  for kc in range(NEC):
                      nc.sync.dma_start(out=wih_sb[:, kc * G4:(kc + 1) * G4], in_=wihT[kc])
                  for kc in range(NKC):
                      nc.sync.dma_start(out=whh_sb[:, kc * G4:(kc + 1) * G4], in_=whhT[kc])
                  h0t = rec.tile([128, 128], BF16, tag="h0t")
                  c_cur = [rec.tile([128, 64], FP32, tag=f"c{hf}", name=f"c{hf}") for hf in range(2)]
                  nc.sync.dma_start(out=h0t[:], in_=h_init[:])
                  for hf in range(2):
                      nc.sync.dma_start(out=c_cur[hf][:], in_=c_init[:, hf * 64:(hf + 1) * 64])
                  h_rhs = lambda st_, kc: (h0t[:, kc * 32:(kc + 1) * 32] if st_ < 0 else
                                           hSB[:, st_ * 128 + kc * 32: st_ * 128 + (kc + 1) * 32])

                  prev_hmuls = []
                  for ck in range(nchunk):
                      # ---- A: gather 1024 tokens, transpose, compute xw chunk ----
                      idx_sb = gath.tile([128, 8], I32, tag="idx")
                      nc.sync.dma_start(
                          out=idx_sb[:],
                          in_=sidx[ck * 1024:(ck + 1) * 1024, 0].rearrange("(g p) -> p g", p=128))
                      xT = xwp.tile([128, NEC * 1024], BF16, tag="xT")
                      for g in range(8):
                          xrow = gath.tile([128, E], BF16, tag="xrow")
                          nc.gpsimd.indirect_dma_start(
                              out=xrow[:], out_offset=None,
                              in_=emb_bf[:],
                              in_offset=bass.IndirectOffsetOnAxis(ap=idx_sb[:, g:g + 1], axis=0),
                          )
                          for kc in range(NEC):
                              nc.sync.dma_start(
                                  out=xT[:, kc * 1024 + g * 128: kc * 1024 + (g + 1) * 128],
                                  in_=xrow[:, kc * 128:(kc + 1) * 128],
                                  transpose=True)
                      # xw chunk: (128p, [l:32][mt:16][b:32]) bf16 (biases folded in)
                      xw = xwp.tile([128, CNK * 512], BF16, tag="xw")
                      xw3 = xw[:].rearrange("p (l m b) -> p l m b", l=CNK, m=NMT)
                      for mt in range(NMT):
                          for nh in range(2):
                              ps = psum.tile([128, 512], FP32, tag="xwps")
                              for kc in range(NEC):
                                  nc.tensor.matmul(
                                      out=ps[:],
                                      lhsT=wih_sb[:, kc * G4 + mt * 128: kc * G4 + (mt + 1) * 128],
                                      rhs=xT[:, kc * 1024 + nh * 512: kc * 1024 + (nh + 1) * 512],
                                      start=(kc == 0), stop=(kc == NEC - 1))
                              # copy+bias into xw with [l][mt][b] interleave
                              cp = nc.vector.tensor_scalar_add(
                                  out=xw3[:, nh * 16:(nh + 1) * 16, mt, :],
                                  in0=ps[:].rearrange("p (l b) -> p l b", l=16),
                                  scalar1=bsum_sb[:, mt:mt + 1])
                              ci = mt * 2 + nh
                              if ci < len(prev_hmuls):
                                  tile.add_dep_helper(
                                      cp.ins, prev_hmuls[ci], sync=False,
                                      reason="xw copy fills post-h-mul DVE gap")

                      # ---- B: 32 recurrence steps ----
                      cur_hmuls = []
                      for sl in range(CNK if "B" in phases else 0):
                          st = ck * CNK + sl
                          gps = psum.tile([128, 512], FP32, tag="gates")
                          xw4 = xw[:].rearrange("p (l m b) -> p l m b", l=CNK, m=NMT)
                          # xw enters psum first via one wide scaled-identity
                          # matmul (no h dependency)
                          nc.tensor.matmul(
                              out=gps[:, 0:512],
                              lhsT=ident_sb[:], rhs=xw4[:, sl, :, :],
                              start=True, stop=False)
                          # mt order in final kc pass: half-0's blocks first
                          mt_half = [[4 * q + j for q in range(4) for j in (0, 1)],
                                     [4 * q + j for q in range(4) for j in (2, 3)]]
                          for kc in range(NKC):
                              mts = (mt_half[0] + mt_half[1]) if kc == NKC - 1 \
                                  else list(range(NMT))
                              rhs = h_rhs(st - 1, kc)
                              for mt in mts:
                                  nc.tensor.matmul(
                                      out=gps[:, mt * 32:(mt + 1) * 32],
                                      lhsT=whh_sb[:, kc * G4 + mt * 128: kc * G4 + (mt + 1) * 128],
                                      rhs=rhs,
                                      start=False, stop=(kc == NKC - 1))
                          # gate math per hidden-half; gate order [i, f, o, g];
                          # ACT reads psum directly with 1/scale applied
                          c_nxt = []
                          gp4 = gps[:].rearrange("p (m b) -> p m b", m=NMT)
                          for hf in range(2):
                              pv = gp4[:, :, :].rearrange("p (g j) b -> p g j b", g=4)[
                                  :, :, 2 * hf:2 * hf + 2, :]
                              sIFO = rec.tile([128, 192], FP32, tag=f"s{hf}")
                              nc.scalar.activation(out=sIFO[:].rearrange("p (g x) -> p g x", g=3),
                                                   in_=pv[:, 0:3], func=AF.Sigmoid,
                                                   scale=w8inv_sb[:, 0:1])
                              gG = rec.tile([128, 64], FP32, tag=f"g{hf}")
                              nc.scalar.activation(out=gG[:].rearrange("p (j b) -> p j b", j=2),
                                                   in_=pv[:, 3], func=AF.Tanh,
                                                   scale=w8inv_sb[:, 0:1])
                              t1 = rec.tile([128, 64], FP32, tag=f"t1{hf}")
                              t2 = rec.tile([128, 64], FP32, tag=f"t2{hf}")
                              nc.vector.tensor_mul(out=t1[:], in0=sIFO[:, 64:128], in1=c_cur[hf][:])
                              nc.vector.tensor_mul(out=t2[:], in0=sIFO[:, 0:64], in1=gG[:])
                              cn = rec.tile([128, 64], FP32, tag=f"c{hf}")
                              nc.vector.tensor_add(out=cn[:], in0=t1[:], in1=t2[:])
                              tch = rec.tile([128, 64], FP32, tag=f"tc{hf}")
                              nc.scalar.activation(out=tch[:], in_=cn[:], func=AF.Tanh)
                              hv = hSB[:, st * 128 + hf * 64: st * 128 + (hf + 1) * 64]
                              hm = nc.vector.tensor_mul(out=hv, in0=sIFO[:, 128:192], in1=tch[:])
                              if hf == 1:
                                  cur_hmuls.append(hm.ins)
                              c_nxt.append(cn)
                          c_cur = c_nxt
                      prev_hmuls = cur_hmuls

              # ============ phase C: projection + AllReduce ============
              if "C" not in phases:
                  continue
              with nc.named_scope("phC"), \
                   tc.tile_pool(name="proj", bufs=3) as proj, \
                   tc.tile_pool(name="projb", bufs=1) as projb, \
                   tc.tile_pool(name="psC", bufs=2, space="PSUM") as psum:
                  wA_sb = projb.tile([128, NKC * T], BF16)
                  wB_sb = projb.tile([128, NKC * T], BF16)
                  for kc in range(NKC):
                      nc.sync.dma_start(out=wA_sb[:, kc * T:(kc + 1) * T], in_=woutA[kc])
                      nc.sync.dma_start(out=wB_sb[:, kc * T:(kc + 1) * T], in_=woutB[kc])
                  ftT_sb = projb.tile([T, ntok], FP32)
                  h3 = hSB[:].rearrange("p (t hc b) -> p t hc b", t=s, hc=NKC)
                  for j in range(ntok // 512):
                      fps = psum.tile([T, 512], FP32, tag="fps")
                      first = True
                      # role A: ascending s blocks, strided SBUF rhs
                      for kc in range(NKC):
                          nc.tensor.matmul(
                              out=fps[:], lhsT=wA_sb[:, kc * T:(kc + 1) * T],
                              rhs=h3[:, j * 16:(j + 1) * 16, kc, :],
                              start=first, stop=False)
                          first = False
                      # role B: this core's storage order is reversed for its
                      # role, so read s descending via a negative-stride AP
                      hi = s - 1 - j * 16
                      lo = hi - 16 if hi - 16 >= 0 else None
                      for kc in range(NKC):
                          nc.tensor.matmul(
                              out=fps[:],
                              lhsT=wB_sb[:, kc * T:(kc + 1) * T],
                              rhs=h3[:, hi:lo:-1, kc, :],
                              start=False,
                              stop=(kc == NKC - 1))
                      nc.scalar.copy(out=ftT_sb[:, j * 512:(j + 1) * 512], in_=fps[:])
                  with nc.named_scope("phR"):
                      nc.sync.dma_start(out=featT_loc[:], in_=ftT_sb[:])
                      if "R" in phases:
                          nc.gpsimd.collective_compute(
                              "AllReduce", ALU.add,
                              replica_groups=[[2 * i, 2 * i + 1] for i in range(n_cores // 2)],
                              ins=[featT_loc.opt()], outs=[featT_shr.opt()])
                      else:
                          nc.sync.dma_start(out=featT_shr[:], in_=featT_loc[:])

              # ============ phase D: CRF forward + emit ============
              if "D" not in phases:
                  continue
              with nc.named_scope("phD"), \
                   tc.tile_pool(name="crf", bufs=2) as crf, \
                   tc.tile_pool(name="crfb", bufs=1) as crfb, \
                   tc.tile_pool(name="psD", bufs=2, space="PSUM") as psum:
                  # featT stays raw: b_out is folded into the CRF per-step
                  # scalar (rn) host-side; emit bias corrected on host
                  featT = crfb.tile([T, ntok], FP32)
                  ef = crfb.tile([T, ntok], FP32)
                  nc.sync.dma_start(out=featT[:], in_=featT_shr[:])
                  for q in range(ntok // 512):
                      sl_ = slice(q * 512, (q + 1) * 512)
                      nc.scalar.activation(out=ef[:, sl_], in_=featT[:, sl_], func=AF.Exp)

                  ET_sb = const.tile([T, T], FP32)
                  rn_sb = const.tile([T, 2], FP32)
                  X0_sb = const.tile([T, B], FP32)
                  es_sb = const.tile([T, 1], FP32)
                  on_sb = const.tile([T, 1], FP32)
                  nc.sync.dma_start(out=ET_sb[:], in_=ET[:])
                  nc.sync.dma_start(out=rn_sb[:], in_=rn[:])
                  nc.sync.dma_start(out=X0_sb[:], in_=X0[:])
                  nc.sync.dma_start(out=es_sb[:], in_=estopT[:])
                  nc.sync.dma_start(out=on_sb[:], in_=onesT[:])

                  X = X0_sb
                  for st in range(s):
                      sps = psum.tile([T, B], FP32, tag="sps")
                      nc.tensor.matmul(out=sps[:], lhsT=ET_sb[:], rhs=X[:], start=True, stop=True)
                      Xn = crf.tile([T, B], FP32, tag="X")
                      rcol = 1 if (st % RENORM_EVERY == RENORM_EVERY - 1) else 0
                      nc.vector.scalar_tensor_tensor(
                          out=Xn[:], in0=sps[:], scalar=rn_sb[:, rcol:rcol + 1],
                          in1=ef[:, st * B:(st + 1) * B], op0=ALU.mult, op1=ALU.mult)
                      X = Xn
                  zps = psum.tile([1, B], FP32, tag="zps")
                  nc.tensor.matmul(out=zps[:], lhsT=es_sb[:], rhs=X[:], start=True, stop=True)
                  z_sb = crf.tile([1, B], FP32, tag="z")
                  nc.scalar.activation(out=z_sb[:], in_=zps[:], func=AF.Ln)
                  nc.sync.dma_start(out=out_z[:], in_=z_sb[:])

                  # gold emit: sum_s featT[tag_{s+1}] via mask multiply-reduce
                  mask_sb = crfb.tile([T, ntok], BF16)
                  nc.sync.dma_start(out=mask_sb[:], in_=gmask[:])
                  nq = ntok // 512
                  estage = crfb.tile([T, nq * B], FP32)
                  for q in range(nq):
                      mprod = crf.tile([T, 512], FP32, tag="mprod")
                      nc.vector.tensor_mul(
                          out=mprod[:], in0=featT[:, q * 512:(q + 1) * 512],
                          in1=mask_sb[:, q * 512:(q + 1) * 512])
                      nc.vector.tensor_reduce(
                          out=estage[:, q * B:(q + 1) * B],
                          in_=mprod[:].rearrange("t (l b) -> t b l", b=B),
                          axis=mybir.AxisListType.X, op=ALU.add)
                  emit2 = crf.tile([T, B], FP32, tag="emit2")
                  nc.vector.tensor_reduce(
                      out=emit2[:], in_=estage[:].rearrange("t (q b) -> t b q", b=B),
                      axis=mybir.AxisListType.X, op=ALU.add)
                  eps = psum.tile([1, B], FP32, tag="eps")
                  nc.tensor.matmul(out=eps[:], lhsT=on_sb[:], rhs=emit2[:], start=True, stop=True)
                  e_sb = crf.tile([1, B], FP32, tag="e")
                  nc.scalar.copy(out=e_sb[:], in_=eps[:])
                  nc.sync.dma_start(out=out_emit[:], in_=e_sb[:])

    nc.compile()
    return nc


def build_null_program(n_cores=8):
    """Same I/O surface, no work — measures pure dispatch overhead."""
    nc = bacc.Bacc("TRN2", target_bir_lowering=False, debug=False,
                   num_devices=n_cores)
    rn = nc.dram_tensor("rn", [T, 2], FP32, kind="ExternalInput")
    out_z = nc.dram_tensor("out_z", [1, B], FP32, kind="ExternalOutput")
    out_emit = nc.dram_tensor("out_emit", [1, B], FP32, kind="ExternalOutput")
    with tile.TileContext(nc) as tc:
        with tc.tile_pool(name="sb", bufs=1) as sb:
            t = sb.tile([1, B], FP32)
            nc.gpsimd.memset(t[:], 0.0)
            nc.sync.dma_start(out=t[:, 0:2], in_=rn[0:1, 0:2])
            nc.sync.dma_start(out=out_z[:], in_=t[:])
            nc.sync.dma_start(out=out_emit[:], in_=t[:])
    nc.compile()
    return nc


# ---------------- host side ----------------

def _prep_inputs(inputs, n_cores=8, s=S, w8=False):
    """Build per-core in_maps from full inputs."""
    f32 = np.float32
    sentence = np.asarray(inputs["sentence"]).astype(np.int32)[:s]   # (s,B)
    tags = np.asarray(inputs["tags"]).astype(np.int64)[:s]
    emb = np.asarray(inputs["emb"], f32)
    trans = np.asarray(inputs["transitions"], f32)
    w_out = np.asarray(inputs["w_out"], f32)
    b_out = np.asarray(inputs["b_out"], f32)
    h0 = np.asarray(inputs["h0"], f32)
    c0 = np.asarray(inputs["c0"], f32)

    bf = ml_dtypes.bfloat16
    emb_bf = emb.astype(bf)

    # gate row permutation [i, f, g, o] -> [i, f, o, g]
    gperm = np.r_[0:2 * H, 3 * H:4 * H, 2 * H:3 * H]
    f8 = ml_dtypes.float8_e4m3fn

    def lstm_pack(wih, whh, bih, bhh):
        """Shared fp8 scale for w_ih and w_hh. xw is stored pre-scaled
        (s*(xw+bias)) so the identity injector is unscaled and the single ACT
        1/s rescale covers both contributions."""
        wih = np.asarray(wih, f32)[gperm]
        whh = np.asarray(whh, f32)[gperm]
        wihT_f = np.ascontiguousarray(wih.T).reshape(NEC, 128, G4)
        whhT_f = np.ascontiguousarray(whh.T).reshape(NKC, 128, G4)
        if w8:
            amax = float(max(np.abs(wih).max(), np.abs(whh).max())) or 1.0
            # keep amax*scale <= 228 < 240 (TRN fp8e4 max normal)
            scale = 2.0 ** int(np.floor(np.log2(228.0 / amax)))
            wihT = (wihT_f * scale).astype(f8)
            whhT = (whhT_f * scale).astype(f8)
        else:
            scale = 1.0
            wihT = wihT_f.astype(bf)
            whhT = whhT_f.astype(bf)
        bs = ((np.asarray(bih, f32)[gperm] + np.asarray(bhh, f32)[gperm])
              * scale).reshape(NMT, 128).T.copy()   # (128, NMT), pre-scaled
        return (wihT, whhT, bs, np.full((128, 1), 1.0 / scale, f32),
                np.eye(128, dtype=f32).astype(bf))

    wihT_f, whhT_f, bs_f, w8i_f, id_f = lstm_pack(inputs["w_ih_f"], inputs["w_hh_f"],
                                            inputs["b_ih_f"], inputs["b_hh_f"])
    wihT_b, whhT_b, bs_b, w8i_b, id_b = lstm_pack(inputs["w_ih_b"], inputs["w_hh_b"],
                                            inputs["b_ih_b"], inputs["b_hh_b"])

    def hc_pack(h, dt):
        # h (B, H) -> (128, [hc:4][b:32]) with hidden hc*128+p at (p, hc*32+b)
        return np.ascontiguousarray(h.T.reshape(NKC, 128, B).transpose(1, 0, 2)
                                    .reshape(128, 128)).astype(dt)

    h_init_f = hc_pack(h0[0], bf); c_init_f = hc_pack(c0[0], f32)
    h_init_b = hc_pack(h0[1], bf); c_init_b = hc_pack(c0[1], f32)

    sidx_f = np.ascontiguousarray(sentence.reshape(s * B, 1))            # tok = st*B+b
    sidx_b = np.ascontiguousarray(sentence[::-1].reshape(s * B, 1))

    woutA = np.ascontiguousarray(w_out[:, :H].T).reshape(NKC, 128, T).astype(bf)
    woutB = np.ascontiguousarray(w_out[:, H:].T).reshape(NKC, 128, T).astype(bf)
    wzero = np.zeros_like(woutA)

    E_mat = np.exp(trans).astype(f32)           # E[t,j] = exp(trans[t,j])
    ET = np.ascontiguousarray(E_mat.T)          # lhsT[j,t]
    X0 = np.zeros((T, B), f32); X0[START, :] = 1.0
    estopT = np.exp(trans[STOP, :]).astype(f32).reshape(T, 1)
    valid = np.arange(T) != START
    c_grow = float(np.log(np.exp(trans[valid]).sum(axis=1)).mean())
    # CRF per-step scalar: exp(b_out) (bias folded here, featT stays raw),
    # col 1 additionally carries the periodic renorm constant
    eb = np.exp(b_out.astype(np.float64)).astype(f32)
    rn = np.stack([eb, eb * np.float32(np.exp(-RENORM_EVERY * c_grow))],
                  axis=1).astype(f32)
    onesT = np.ones((T, 1), f32)
    boutT = b_out.astype(f32).reshape(T, 1)

    # gold mask + host-side pure-index scores
    tags_b = tags.T                                   # (B,s)
    tags_ext = np.concatenate([np.full((B, 1), START, tags_b.dtype), tags_b], axis=1)
    t_prev, t_next = tags_ext[:, :-1], tags_ext[:, 1:]
    trans_sc = trans[t_next, t_prev].sum(axis=1) + trans[STOP, tags_ext[:, -1]] \
        + np.asarray(inputs["b_out"], f32)[t_next].sum(axis=1)   # (B,) + emit bias
    gmask = np.zeros((T, s * B), f32)
    st_idx = np.repeat(np.arange(s), B)
    b_idx = np.tile(np.arange(B), s)
    gmask[tags.reshape(-1), st_idx * B + b_idx] = 1.0

    common = dict(emb_bf=emb_bf, bout=boutT, ET=ET, X0=X0, estopT=estopT,
                  rn=rn, onesT=onesT, gmask=gmask)
    in_maps = []
    for core in range(n_cores):
        if core == 1:
            m = dict(common, sidx=sidx_b, wihT=wihT_b, whhT=whhT_b, bsum=bs_b, w8inv=w8i_b, ident=id_b,
                     h_init=h_init_b, c_init=c_init_b, woutA=wzero, woutB=woutB)
        else:
            m = dict(common, sidx=sidx_f, wihT=wihT_f, whhT=whhT_f, bsum=bs_f, w8inv=w8i_f, ident=id_f,
                     h_init=h_init_f, c_init=c_init_f,
                     woutA=(woutA if core == 0 else wzero), woutB=wzero)
        in_maps.append(m)

    n_renorm = sum(1 for st in range(s) if st % RENORM_EVERY == RENORM_EVERY - 1)
    host = dict(trans_sc=trans_sc, corr=n_renorm * RENORM_EVERY * c_grow)
    return in_maps, host


def assemble_loss(res0, host):
    fwd = res0["out_z"][0].astype(np.float64) + host["corr"]
    gold = res0["out_emit"][0].astype(np.float64) + host["trans_sc"]
    return np.float32((fwd - gold).sum())


_CACHE = {}


W8_DEFAULT = True


def kernel(**inputs) -> np.ndarray:
    n_cores = 8
    if "nc" not in _CACHE:
        _CACHE["nc"] = build_program(n_cores=n_cores, w8=W8_DEFAULT)
    in_maps, host = _prep_inputs(inputs, n_cores=n_cores, w8=W8_DEFAULT)
    res = run_bass_kernel_spmd(_CACHE["nc"], in_maps, list(range(n_cores)))
    return assemble_loss(res.results[0], host)

